# revision 18
# baseline (speedup 1.0000x reference)
"""DIEN layer (GRU + attention + AUGRU) Trainium2 Bass kernel.

Pure data parallel across 8 NeuronCores: batch 2048 -> 256 per core.

Device layout convention: features on SBUF partitions, batch on the free
dim.  All matmuls keep state in [feat, batch] layout so the recurrence
never transposes.  Ragged sequence handling: for t >= seq_len(b) the
update gate is saturated (v = 1-u -> 0) by adding -BIG to the (negated)
u-gate preactivation via a K=1 matmul, which freezes h exactly; the
attention softmax masks dead positions to exp(NEG-max) = 0, so alphas
are exactly 0 there and the AUGRU also freezes.
"""

import os
import sys

sys.path.insert(0, "/opt/trn_rl_repo")

import numpy as np

import concourse.bass as bass
import concourse.bacc as bacc
import concourse.mybir as mybir
import concourse.tile as tile
from concourse.bass_utils import run_bass_kernel_spmd

B, T, D, H = 2048, 200, 128, 128
NCORES = 8
BL = B // NCORES

BIG = 30000.0
NEG = np.float32(-(2.0**32) + 1.0)

F32 = mybir.dt.float32
F32R = mybir.dt.float32r


def _r(ap):
    """View an fp32 AP as float32r so the PE runs at full rate (N>=256)."""
    return ap.bitcast(F32R)


def build_program(T_=T, BL_=BL, mm_f32r=False):
    """Build the single-core program (SPMD across 8 cores)."""
    nc = bacc.Bacc("TRN2", target_bir_lowering=False, debug=False)
    dt = F32

    def mmcast(ap):
        return _r(ap) if mm_f32r else ap

    # ---- external inputs (per core) ----
    xT = nc.dram_tensor("xT", [T_, D, BL_], dt, kind="ExternalInput").ap()
    qT = nc.dram_tensor("qT", [D, BL_], dt, kind="ExternalInput").ap()
    qN = nc.dram_tensor("qN", [BL_, D], dt, kind="ExternalInput").ap()
    hsum = nc.dram_tensor("hsum", [BL_, D], dt, kind="ExternalInput").ap()
    validBT = nc.dram_tensor("validBT", [BL_, T_], dt, kind="ExternalInput").ap()
    negmBT = nc.dram_tensor("negmBT", [BL_, T_], dt, kind="ExternalInput").ap()
    deadT = nc.dram_tensor("deadT", [T_, BL_], dt, kind="ExternalInput").ap()

    # weights (preprocessed on host; u-halves of gate kernels negated)
    w1x = nc.dram_tensor("w1x", [D, 2 * H], dt, kind="ExternalInput").ap()
    w1h = nc.dram_tensor("w1h", [H, 2 * H], dt, kind="ExternalInput").ap()
    c1x = nc.dram_tensor("c1x", [D, H], dt, kind="ExternalInput").ap()
    c1h = nc.dram_tensor("c1h", [H, H], dt, kind="ExternalInput").ap()
    gb1n = nc.dram_tensor("gb1n", [2 * H], dt, kind="ExternalInput").ap()
    cb1 = nc.dram_tensor("cb1", [H], dt, kind="ExternalInput").ap()
    w2x = nc.dram_tensor("w2x", [H, 2 * H], dt, kind="ExternalInput").ap()
    w2h = nc.dram_tensor("w2h", [H, 2 * H], dt, kind="ExternalInput").ap()
    c2x = nc.dram_tensor("c2x", [H, H], dt, kind="ExternalInput").ap()
    c2h = nc.dram_tensor("c2h", [H, H], dt, kind="ExternalInput").ap()
    gb2n = nc.dram_tensor("gb2n", [2 * H], dt, kind="ExternalInput").ap()
    cb2 = nc.dram_tensor("cb2", [H], dt, kind="ExternalInput").ap()

    wq = nc.dram_tensor("wq", [D, H], dt, kind="ExternalInput").ap()
    bq = nc.dram_tensor("bq", [H], dt, kind="ExternalInput").ap()
    pra = nc.dram_tensor("pra", [H], dt, kind="ExternalInput").ap()
    pra1m = nc.dram_tensor("pra1m", [H], dt, kind="ExternalInput").ap()
    w1apc = nc.dram_tensor("w1apc", [H, 80], dt, kind="ExternalInput").ap()
    w1bmc = nc.dram_tensor("w1bmc", [H, 80], dt, kind="ExternalInput").ap()
    w1d = nc.dram_tensor("w1d", [H, 80], dt, kind="ExternalInput").ap()
    b1 = nc.dram_tensor("b1", [80], dt, kind="ExternalInput").ap()
    w2a = nc.dram_tensor("w2a", [80, 40], dt, kind="ExternalInput").ap()
    b2 = nc.dram_tensor("b2", [40], dt, kind="ExternalInput").ap()
    w3 = nc.dram_tensor("w3", [40, 1], dt, kind="ExternalInput").ap()
    ident = nc.dram_tensor("ident", [128, 128], dt, kind="ExternalInput").ap()
    negbig = nc.dram_tensor("negbig", [1, 128], dt, kind="ExternalInput").ap()
    onescol = nc.dram_tensor("onescol", [1, 128], dt, kind="ExternalInput").ap()

    out = nc.dram_tensor("out", [BL_, 3 * D + H], dt, kind="ExternalOutput").ap()

    # DRAM scratch
    rnn1 = nc.dram_tensor("rnn1", [T_, H, BL_], dt).ap()
    aTd = nc.dram_tensor("aTd", [T_, BL_], dt).ap()

    DCH = 8  # steps of deadrow/alpha rows per [1, DCH*BL] chunk
    n_dch = (T_ + DCH - 1) // DCH

    SIG = mybir.ActivationFunctionType.Sigmoid
    TANH = mybir.ActivationFunctionType.Tanh
    EXP = mybir.ActivationFunctionType.Exp
    RELU = mybir.ActivationFunctionType.Relu
    AX = mybir.AxisListType.X
    MUL = mybir.AluOpType.mult
    SUB = mybir.AluOpType.subtract
    ADDOP = mybir.AluOpType.add
    MAXOP = mybir.AluOpType.max

    with tile.TileContext(nc) as tc:
        # ------- persistent constants in SBUF -------
        with tc.tile_pool(name="wts", bufs=1) as wp:

            def load_w(ap, shape, tag, col=False):
                t_ = wp.tile(shape, dt, tag=tag)
                if col:
                    # 1-D [n] -> SBUF [n<=128, 1] column; n>128 folds to [128, n/128]
                    n = ap.shape[0]
                    if n <= 128:
                        nc.sync.dma_start(t_[:, 0:1], ap.rearrange("(h a) -> h a", a=1))
                    else:
                        nc.sync.dma_start(t_[:], ap.rearrange("(a h) -> h a", h=128))
                else:
                    nc.sync.dma_start(t_[:], ap)
                return t_

            W1x = load_w(w1x, [D, 2 * H], "W1x")
            W1h = load_w(w1h, [H, 2 * H], "W1h")
            C1x = load_w(c1x, [D, H], "C1x")
            C1h = load_w(c1h, [H, H], "C1h")
            W2x = load_w(w2x, [H, 2 * H], "W2x")
            W2h = load_w(w2h, [H, 2 * H], "W2h")
            C2x = load_w(c2x, [H, H], "C2x")
            C2h = load_w(c2h, [H, H], "C2h")
            GB1 = load_w(gb1n, [128, 2], "GB1", col=True)
            CB1 = load_w(cb1, [H, 1], "CB1", col=True)
            GB2 = load_w(gb2n, [128, 2], "GB2", col=True)
            CB2 = load_w(cb2, [H, 1], "CB2", col=True)
            WQ = load_w(wq, [D, H], "WQ")
            BQ = load_w(bq, [H, 1], "BQ", col=True)
            PRA = load_w(pra, [H, 1], "PRA", col=True)
            PRA1M = load_w(pra1m, [H, 1], "PRA1M", col=True)
            W1APC = load_w(w1apc, [H, 80], "W1APC")
            W1BMC = load_w(w1bmc, [H, 80], "W1BMC")
            W1D = load_w(w1d, [H, 80], "W1D")
            B1 = load_w(b1, [80, 1], "B1", col=True)
            W2A = load_w(w2a, [80, 40], "W2A")
            B2 = load_w(b2, [40, 1], "B2", col=True)
            W3 = load_w(w3, [40, 1], "W3")
            IDN = load_w(ident, [128, 128], "IDN")
            NBIG = load_w(negbig, [1, 128], "NBIG")
            ONEC = load_w(onescol, [1, 128], "ONEC")

            # qT / validBT / negmBT stay resident
            QT = wp.tile([D, BL_], dt, tag="QT")
            nc.sync.dma_start(QT[:], qT)
            nbh = (BL_ + 127) // 128
            VAL = []
            NEGM = []
            for i in range(nbh):
                p = min(128, BL_ - i * 128)
                v_ = wp.tile([128, T_], dt, tag=f"VAL{i}")
                nc.sync.dma_start(v_[0:p, :], validBT[i * 128 : i * 128 + p, :])
                VAL.append(v_)
                n_ = wp.tile([128, T_], dt, tag=f"NEGM{i}")
                nc.sync.dma_start(n_[0:p, :], negmBT[i * 128 : i * 128 + p, :])
                NEGM.append(n_)

            # ------- attention query path (once) -------
            with (
                tc.tile_pool(name="ps_small", bufs=2, space="PSUM") as psp,
                tc.tile_pool(name="setup_tmp", bufs=2) as stp,
            ):
                # qp = prelu(wq.T @ qT + bq)
                p_qp = psp.tile([H, BL_], dt, tag="p_qp")
                nc.tensor.matmul(p_qp[:], mmcast(WQ[:]), mmcast(QT[:]), start=True, stop=True)
                # add bias via activation identity, then prelu decomposition:
                # prelu(x) = alpha*x + (1-alpha)*relu(x)
                qpre = stp.tile([H, BL_], dt, tag="qpre")
                nc.scalar.add(qpre[:], p_qp[:], BQ[:, 0:1])
                r1 = stp.tile([H, BL_], dt, tag="r1")
                nc.scalar.activation(r1[:], qpre[:], RELU, scale=PRA1M[:, 0:1])
                QP = wp.tile([H, BL_], dt, tag="QP")
                nc.vector.scalar_tensor_tensor(
                    QP[:], qpre[:], PRA[:, 0:1], r1[:], op0=MUL, op1=ADDOP
                )
                # qc = w1apc.T @ qp + b1  [80, BL]
                p_qc = psp.tile([80, BL_], dt, tag="p_qc")
                nc.tensor.matmul(p_qc[:], mmcast(W1APC[:]), mmcast(QP[:]), start=True, stop=True)
                QC = wp.tile([80, BL_], dt, tag="QC")
                nc.scalar.add(QC[:], p_qc[:], B1[:, 0:1])

            # =================== GRU 1 ===================
            def gru_pass(
                W_x, W_h, C_x, C_h, GBn, CBc, x_of_t, store_rnn1, use_alpha,
                hout=None,
            ):
                """One full recurrence over T_ steps. Copies final h to hout."""
                with (
                    tc.tile_pool(name="g_x", bufs=4) as xp,
                    tc.tile_pool(name="g_h", bufs=3) as hp,
                    tc.tile_pool(name="g_rv", bufs=2) as rvp,
                    tc.tile_pool(name="g_tmp", bufs=4) as tp,
                    tc.tile_pool(name="g_dead", bufs=3) as dp,
                    tc.tile_pool(name="g_pg", bufs=2, space="PSUM") as pgp,
                    tc.tile_pool(name="g_pc", bufs=2, space="PSUM") as pcp,
                    tc.tile_pool(name="g_pa", bufs=2, space="PSUM") as pap,
                ):
                    h = hp.tile([H, BL_], dt, tag="h")
                    nc.vector.memset(h[:], 0.0)
                    dead_ch = None
                    alpha_ch = None
                    for t in range(T_):
                        j = t % DCH
                        if j == 0:
                            n_in = min(DCH, T_ - t)
                            dead_ch = dp.tile([1, DCH * BL_], dt, tag="dead")
                            nc.sync.dma_start(
                                dead_ch[0:1, 0 : n_in * BL_],
                                deadT[t : t + n_in, :].rearrange(
                                    "(c a) b -> c (a b)", c=1
                                ),
                            )
                            if use_alpha:
                                alpha_ch = dp.tile([1, DCH * BL_], dt, tag="alpha")
                                nc.sync.dma_start(
                                    alpha_ch[0:1, 0 : n_in * BL_],
                                    aTd[t : t + n_in, :].rearrange(
                                        "(c a) b -> c (a b)", c=1
                                    ),
                                )
                        x_t = x_of_t(xp, t)
                        # gates: psum [128, 2BL] = [r | v]
                        pg = pgp.tile([128, 2 * BL_], dt, tag="pg")
                        nc.tensor.matmul(
                            pg[:, 0:BL_], mmcast(W_x[:, 0:H]), mmcast(x_t[:]),
                            start=True, stop=False,
                        )
                        nc.tensor.matmul(
                            pg[:, 0:BL_], mmcast(W_h[:, 0:H]), mmcast(h[:]),
                            start=False, stop=True,
                        )
                        nc.tensor.matmul(
                            pg[:, BL_ : 2 * BL_], mmcast(W_x[:, H : 2 * H]), mmcast(x_t[:]),
                            start=True, stop=False,
                        )
                        nc.tensor.matmul(
                            pg[:, BL_ : 2 * BL_], mmcast(W_h[:, H : 2 * H]), mmcast(h[:]),
                            start=False, stop=False,
                        )
                        nc.tensor.matmul(
                            pg[:, BL_ : 2 * BL_],
                            mmcast(NBIG[:]),
                            mmcast(dead_ch[0:1, j * BL_ : (j + 1) * BL_]),
                            start=False, stop=True,
                        )
                        # sigmoid over r and v (separate per-feature biases)
                        rv = rvp.tile([128, 2 * BL_], dt, tag="rv")
                        nc.scalar.activation(rv[:, 0:BL_], pg[:, 0:BL_], SIG, bias=GBn[:, 0:1])
                        nc.scalar.activation(
                            rv[:, BL_ : 2 * BL_], pg[:, BL_ : 2 * BL_], SIG,
                            bias=GBn[:, 1:2],
                        )
                        # candidate
                        rh = tp.tile([H, BL_], dt, tag="rh")
                        nc.vector.tensor_mul(rh[:], rv[:, 0:BL_], h[:])
                        pc = pcp.tile([H, BL_], dt, tag="pc")
                        nc.tensor.matmul(pc[:], mmcast(C_x[:]), mmcast(x_t[:]), start=True, stop=False)
                        nc.tensor.matmul(pc[:], mmcast(C_h[:]), mmcast(rh[:]), start=False, stop=True)
                        c = tp.tile([H, BL_], dt, tag="c")
                        nc.scalar.activation(c[:], pc[:], TANH, bias=CBc[:, 0:1])
                        # v_eff
                        if use_alpha:
                            pa = pap.tile([128, BL_], dt, tag="pa")
                            nc.tensor.matmul(
                                pa[:], mmcast(ONEC[:]),
                                mmcast(alpha_ch[0:1, j * BL_ : (j + 1) * BL_]),
                                start=True, stop=True,
                            )
                            t1 = tp.tile([H, BL_], dt, tag="t1")
                            nc.vector.scalar_tensor_tensor(
                                t1[:], rv[:, BL_ : 2 * BL_], 1.0, pa[:],
                                op0=SUB, op1=MUL,
                            )
                            veff = tp.tile([H, BL_], dt, tag="veff")
                            nc.vector.tensor_sub(veff[:], rv[:, BL_ : 2 * BL_], t1[:])
                        else:
                            veff = rv[:, BL_ : 2 * BL_]
                        # h' = h + veff*(c - h)
                        d_ = tp.tile([H, BL_], dt, tag="d_")
                        nc.vector.tensor_sub(d_[:], c[:], h[:])
                        e_ = tp.tile([H, BL_], dt, tag="e_")
                        nc.vector.tensor_mul(e_[:], veff[:], d_[:])
                        h2 = hp.tile([H, BL_], dt, tag="h")
                        nc.vector.tensor_add(h2[:], h[:], e_[:])
                        if store_rnn1:
                            nc.sync.dma_start(rnn1[t], h2[:])
                        h = h2
                    if hout is not None:
                        nc.vector.tensor_copy(hout[:], h[:])

            def x_from_xT(xp, t):
                x_t = xp.tile([D, BL_], dt, tag="x")
                nc.sync.dma_start(x_t[:], xT[t])
                return x_t

            hfin = wp.tile([H, BL_], dt, tag="hfin")
            gru_pass(W1x, W1h, C1x, C1h, GB1, CB1, x_from_xT, True, False)

            # =================== attention scores ===================
            rnn1_p = rnn1.rearrange("t p b -> p t b")
            TCH = max(1, 512 // BL_)  # time steps per chunk (free dim 512)
            n_tch = (T_ + TCH - 1) // TCH
            with (
                tc.tile_pool(name="a_sc", bufs=1, space="PSUM") as scp,
                tc.tile_pool(name="a_in", bufs=4) as ainp,
                tc.tile_pool(name="a_tmp", bufs=4) as atp,
            ):
                p_sc = [
                    scp.tile([128, T_], dt, tag=f"p_sc{i}", name=f"p_sc{i}")
                    for i in range(nbh)
                ]
                app_cm = tc.tile_pool(name="a_ps", bufs=2, space="PSUM")
                app = app_cm.__enter__()
                for ci in range(n_tch):
                    t0 = ci * TCH
                    nt = min(TCH, T_ - t0)
                    w_ = nt * BL_
                    rc = ainp.tile([H, TCH * BL_], dt, tag="rc")
                    rc3 = rc[:].rearrange("p (a b) -> p a b", a=TCH)[:, 0:nt, :]
                    nc.sync.dma_start(rc3, rnn1_p[:, t0 : t0 + nt, :])
                    qp_bc = (
                        QP[:]
                        .rearrange("p (a b) -> p a b", a=1)
                        .broadcast_to([H, nt, BL_])
                    )
                    prod = atp.tile([H, TCH * BL_], dt, tag="prod")
                    prod3 = prod[:].rearrange("p (a b) -> p a b", a=TCH)[:, 0:nt, :]
                    nc.vector.tensor_mul(prod3, rc3, qp_bc)
                    p_s1 = app.tile([80, TCH * BL_], dt, tag="p_s1")
                    nc.tensor.matmul(
                        p_s1[:, 0:w_], mmcast(W1BMC[:]), mmcast(rc[:, 0:w_]),
                        start=True, stop=False,
                    )
                    nc.tensor.matmul(
                        p_s1[:, 0:w_], mmcast(W1D[:]), mmcast(prod[:, 0:w_]),
                        start=False, stop=True,
                    )
                    qc_bc = (
                        QC[:]
                        .rearrange("p (a b) -> p a b", a=1)
                        .broadcast_to([80, nt, BL_])
                    )
                    a1 = atp.tile([80, TCH * BL_], dt, tag="a1")
                    nc.vector.tensor_add(
                        a1[:].rearrange("p (a b) -> p a b", a=TCH)[:, 0:nt, :],
                        p_s1[:].rearrange("p (a b) -> p a b", a=TCH)[:, 0:nt, :],
                        qc_bc,
                    )
                    a1s = atp.tile([80, TCH * BL_], dt, tag="a1s")
                    nc.scalar.activation(a1s[:, 0:w_], a1[:, 0:w_], SIG)
                    p_s2 = app.tile([40, TCH * BL_], dt, tag="p_s2")
                    nc.tensor.matmul(
                        p_s2[:, 0:w_], mmcast(W2A[:]), mmcast(a1s[:, 0:w_]),
                        start=True, stop=True,
                    )
                    a2 = atp.tile([40, TCH * BL_], dt, tag="a2")
                    nc.scalar.activation(a2[:, 0:w_], p_s2[:, 0:w_], SIG, bias=B2[:, 0:1])
                    for k in range(nt):
                        t = t0 + k
                        for i in range(nbh):
                            p = min(128, BL_ - i * 128)
                            nc.tensor.matmul(
                                p_sc[i][0:p, t : t + 1],
                                mmcast(a2[:, k * BL_ + i * 128 : k * BL_ + i * 128 + p]),
                                mmcast(W3[:]),
                                start=True, stop=True,
                            )
                app_cm.__exit__(None, None, None)
                # masked softmax over T per sample; alphas -> aTd (transposed)
                with (
                    tc.tile_pool(name="a_sm", bufs=1) as smp,
                    tc.tile_pool(name="a_tr", bufs=2, space="PSUM") as trp,
                ):
                    for i in range(nbh):
                        p = min(128, BL_ - i * 128)
                        sm = smp.tile([128, T_], dt, tag=f"sm{i}")
                        nc.vector.scalar_tensor_tensor(
                            sm[0:p, :], p_sc[i][0:p, :], 1.0, VAL[i][0:p, :],
                            op0=MUL, op1=MUL,
                        )
                        msk = smp.tile([128, T_], dt, tag=f"msk{i}")
                        nc.vector.tensor_add(msk[0:p, :], sm[0:p, :], NEGM[i][0:p, :])
                        nmx = smp.tile([128, 1], dt, tag=f"nmx{i}")
                        nc.vector.tensor_reduce(
                            nmx[0:p, :], msk[0:p, :], axis=AX, op=MAXOP, negate=True
                        )
                        ex = smp.tile([128, T_], dt, tag=f"ex{i}")
                        nc.scalar.activation(ex[0:p, :], msk[0:p, :], EXP, bias=nmx[0:p, 0:1])
                        sume = smp.tile([128, 1], dt, tag=f"sume{i}")
                        nc.vector.tensor_reduce(
                            sume[0:p, :], ex[0:p, :], axis=AX, op=ADDOP
                        )
                        rec = smp.tile([128, 1], dt, tag=f"rec{i}")
                        nc.vector.reciprocal(rec[0:p, :], sume[0:p, :])
                        alp = smp.tile([128, T_], dt, tag=f"alp{i}")
                        nc.vector.tensor_scalar_mul(alp[0:p, :], ex[0:p, :], rec[0:p, 0:1])
                        # transpose alphas [p, T] -> aTd[:, i*128 : ...]
                        for c0 in range(0, T_, 128):
                            w2_ = min(128, T_ - c0)
                            pt = trp.tile([128, 128], dt, tag="p_tr")
                            nc.tensor.transpose(
                                pt[0:w2_, 0:p], alp[0:p, c0 : c0 + w2_], IDN[0:p, 0:p]
                            )
                            st = smp.tile([128, 128], dt, tag="st")
                            nc.scalar.copy(st[0:w2_, 0:p], pt[0:w2_, 0:p])
                            nc.sync.dma_start(
                                aTd[c0 : c0 + w2_, i * 128 : i * 128 + p],
                                st[0:w2_, 0:p],
                            )

            # =================== AUGRU ===================
            def x_from_rnn1(xp, t):
                x_t = xp.tile([H, BL_], dt, tag="x")
                nc.sync.dma_start(x_t[:], rnn1[t])
                return x_t

            gru_pass(W2x, W2h, C2x, C2h, GB2, CB2, x_from_rnn1, False, True, hout=hfin)

            # =================== output assembly ===================
            with (
                tc.tile_pool(name="o_t", bufs=2) as otp,
                tc.tile_pool(name="o_p", bufs=2, space="PSUM") as opp,
            ):
                for i in range(nbh):
                    p = min(128, BL_ - i * 128)
                    qs = otp.tile([128, D], dt, tag="qs")
                    nc.sync.dma_start(qs[0:p, :], qN[i * 128 : i * 128 + p, :])
                    hs = otp.tile([128, D], dt, tag="hs")
                    nc.sync.dma_start(hs[0:p, :], hsum[i * 128 : i * 128 + p, :])
                    pr = otp.tile([128, D], dt, tag="pr")
                    nc.vector.tensor_mul(pr[0:p, :], qs[0:p, :], hs[0:p, :])
                    nc.sync.dma_start(out[i * 128 : i * 128 + p, 0:D], qs[0:p, :])
                    nc.sync.dma_start(out[i * 128 : i * 128 + p, D : 2 * D], hs[0:p, :])
                    nc.sync.dma_start(out[i * 128 : i * 128 + p, 2 * D : 3 * D], pr[0:p, :])
                    ptr = opp.tile([128, 128], dt, tag="ptr")
                    nc.tensor.transpose(
                        ptr[0:p, :], hfin[:, i * 128 : i * 128 + p], IDN[:]
                    )
                    ht = otp.tile([128, H], dt, tag="ht")
                    nc.scalar.copy(ht[0:p, :], ptr[0:p, :])
                    nc.sync.dma_start(
                        out[i * 128 : i * 128 + p, 3 * D : 3 * D + H], ht[0:p, :]
                    )

    nc.compile()
    return nc


def host_prep(item_eb, item_his_eb, item_his_eb_sum, mask,
              gk1, gb1, ck1, cb1,
              wq, bq, prelu_alpha, w1, b1, w2, b2, w3, b3,
              gk2, gb2, ck2, cb2, T_=T, BL_=BL, ncores=NCORES):
    """Shard + preprocess inputs into per-core in_maps."""
    f = np.float32
    # shared weight transforms
    w1x = np.ascontiguousarray(gk1[:D]).astype(f)
    w1h = np.ascontiguousarray(gk1[D:]).astype(f)
    w1x[:, H:] = -w1x[:, H:]
    w1h[:, H:] = -w1h[:, H:]
    gb1n = gb1.astype(f).copy()
    gb1n[H:] = -gb1n[H:]
    w2x_ = np.ascontiguousarray(gk2[:H]).astype(f)
    w2h_ = np.ascontiguousarray(gk2[H:]).astype(f)
    w2x_[:, H:] = -w2x_[:, H:]
    w2h_[:, H:] = -w2h_[:, H:]
    gb2n = gb2.astype(f).copy()
    gb2n[H:] = -gb2n[H:]
    shared = dict(
        w1x=w1x, w1h=w1h, c1x=np.ascontiguousarray(ck1[:D]).astype(f),
        c1h=np.ascontiguousarray(ck1[D:]).astype(f), gb1n=gb1n, cb1=cb1.astype(f),
        w2x=w2x_, w2h=w2h_, c2x=np.ascontiguousarray(ck2[:H]).astype(f),
        c2h=np.ascontiguousarray(ck2[H:]).astype(f), gb2n=gb2n, cb2=cb2.astype(f),
        wq=wq.astype(f), bq=bq.astype(f), pra=prelu_alpha.astype(f),
        pra1m=(1.0 - prelu_alpha).astype(f),
        w1apc=np.ascontiguousarray(w1[:H] + w1[2 * H : 3 * H]).astype(f),
        w1bmc=np.ascontiguousarray(w1[H : 2 * H] - w1[2 * H : 3 * H]).astype(f),
        w1d=np.ascontiguousarray(w1[3 * H :]).astype(f),
        b1=b1.astype(f), w2a=w2.astype(f), b2=b2.astype(f), w3=w3.astype(f),
        ident=np.eye(128, dtype=f), negbig=np.full((1, 128), -BIG, f),
        onescol=np.ones((1, 128), f),
    )
    in_maps = []
    for c in range(ncores):
        sl = slice(c * BL_, (c + 1) * BL_)
        m = mask[sl]
        has0 = (m == 0).any(axis=1)
        ln = np.where(has0, np.argmax(m == 0, axis=1), T_).astype(np.int64)
        tt = np.arange(T_)
        valid = (tt[None, :] < ln[:, None])
        im = dict(shared)
        im["xT"] = np.ascontiguousarray(
            item_his_eb[sl].transpose(1, 2, 0)).astype(f)
        im["qT"] = np.ascontiguousarray(item_eb[sl, 0].T).astype(f)
        im["qN"] = np.ascontiguousarray(item_eb[sl, 0]).astype(f)
        im["hsum"] = np.ascontiguousarray(item_his_eb_sum[sl]).astype(f)
        im["validBT"] = valid.astype(f)
        im["negmBT"] = np.where(valid, 0.0, NEG).astype(f)
        im["deadT"] = np.ascontiguousarray((~valid).T).astype(f)
        in_maps.append(im)
    return in_maps


_prog_cache = {}


def kernel(**inputs):
    key = "full"
    if key not in _prog_cache:
        _prog_cache[key] = build_program()
    nc = _prog_cache[key]
    in_maps = host_prep(**inputs)
    res = run_bass_kernel_spmd(nc, in_maps, list(range(NCORES)))
    return np.concatenate([res.results[c]["out"] for c in range(NCORES)], axis=0)


# revision 26
# speedup vs baseline: 4520.1672x; 4520.1672x over previous
"""DIEN layer (GRU + attention + AUGRU) Trainium2 Bass kernel.

Pure data parallel across 8 NeuronCores: batch 2048 -> 256 per core.

Device layout: features on SBUF partitions, batch on the free dim; all
matmuls keep state in [feat, batch] layout so the recurrence never
transposes.  Ragged sequences: for t >= seq_len(b) the update gate is
saturated (v = 1-u -> 0) by adding -BIG to the (negated) u-gate
preactivation via a K=1 matmul, which freezes h exactly; the attention
softmax masks dead positions to exp(NEG-max) = 0 so alphas are exactly
0 there and the AUGRU also freezes.  Compute dtype for matmuls and
elementwise is bf16 (fp32 PSUM accumulation, fp32 softmax); set
dtc_name="f32" for a full-precision (4x slower matmul) variant.
"""

import sys

sys.path.insert(0, "/opt/trn_rl_repo")

import numpy as np
import ml_dtypes

import concourse.bacc as bacc
import concourse.mybir as mybir
import concourse.tile as tile
from concourse.bass_utils import run_bass_kernel_spmd

B, T, D, H = 2048, 200, 128, 128
NCORES = 8
BL = B // NCORES

BIG = 30000.0
NEG = np.float32(-(2.0**32) + 1.0)

F32 = mybir.dt.float32
BF16 = mybir.dt.bfloat16


def build_program(T_=T, BL_=BL, dtc_name="bf16"):
    """Build the single-core program (run SPMD across 8 cores)."""
    nc = bacc.Bacc("TRN2", target_bir_lowering=False, debug=False)
    dt = F32
    dtc = BF16 if dtc_name == "bf16" else F32

    def dram(name, shape, dty=dt, kind="ExternalInput"):
        return nc.dram_tensor(name, shape, dty, kind=kind).ap()

    # ---- external inputs (per core); compute-dtype tensors use dtc ----
    xT = dram("xT", [T_, D, BL_], dtc)
    qT = dram("qT", [D, BL_], dtc)
    qN = dram("qN", [BL_, D])
    hsum = dram("hsum", [BL_, D])
    validBT = dram("validBT", [BL_, T_])
    negmBT = dram("negmBT", [BL_, T_])
    deadT = dram("deadT", [T_, BL_], dtc)

    w1x = dram("w1x", [D, 2 * H], dtc)
    w1h = dram("w1h", [H, 2 * H], dtc)
    c1x = dram("c1x", [D, H], dtc)
    c1h = dram("c1h", [H, H], dtc)
    gb1n = dram("gb1n", [2 * H])
    cb1 = dram("cb1", [H])
    w2x = dram("w2x", [H, 2 * H], dtc)
    w2h = dram("w2h", [H, 2 * H], dtc)
    c2x = dram("c2x", [H, H], dtc)
    c2h = dram("c2h", [H, H], dtc)
    gb2n = dram("gb2n", [2 * H])
    cb2 = dram("cb2", [H])

    wq = dram("wq", [D, H], dtc)
    bq = dram("bq", [H])
    pra = dram("pra", [H])
    pra1m = dram("pra1m", [H])
    w1apc = dram("w1apc", [H, 80], dtc)
    w1bmc = dram("w1bmc", [H, 80], dtc)
    w1d = dram("w1d", [H, 80], dtc)
    b1 = dram("b1", [80])
    w2a = dram("w2a", [80, 40], dtc)
    b2 = dram("b2", [40])
    w3 = dram("w3", [40, 1], dtc)
    ident = dram("ident", [128, 128])
    identc = dram("identc", [128, 128], dtc)
    negbig = dram("negbig", [1, 128], dtc)
    onescol = dram("onescol", [1, 128], dtc)

    out = dram("out", [BL_, 3 * D + H], dt, kind="ExternalOutput")

    # DRAM scratch: rnn1 outputs and (1 - alpha) rows, compute dtype
    rnn1 = nc.dram_tensor("rnn1", [T_, H, BL_], dtc).ap()
    aTd = nc.dram_tensor("aTd", [T_, BL_], dtc).ap()

    DCH = 8  # steps of deadrow/alpha rows per [1, DCH*BL] chunk

    SIG = mybir.ActivationFunctionType.Sigmoid
    TANH = mybir.ActivationFunctionType.Tanh
    EXP = mybir.ActivationFunctionType.Exp
    RELU = mybir.ActivationFunctionType.Relu
    COPYF = mybir.ActivationFunctionType.Copy
    AX = mybir.AxisListType.X
    MUL = mybir.AluOpType.mult
    SUB = mybir.AluOpType.subtract
    ADDOP = mybir.AluOpType.add
    MAXOP = mybir.AluOpType.max

    with tile.TileContext(nc) as tc:
        with tc.tile_pool(name="wts", bufs=1) as wp:

            def load_w(ap, shape, tag, col=False, dty=dtc):
                t_ = wp.tile(shape, dty, tag=tag, name=tag)
                if col:
                    n = ap.shape[0]
                    if n <= 128:
                        nc.sync.dma_start(t_[:, 0:1], ap.rearrange("(h a) -> h a", a=1))
                    else:
                        nc.sync.dma_start(t_[:], ap.rearrange("(a h) -> h a", h=128))
                else:
                    nc.sync.dma_start(t_[:], ap)
                return t_

            W1x = load_w(w1x, [D, 2 * H], "W1x")
            W1h = load_w(w1h, [H, 2 * H], "W1h")
            C1x = load_w(c1x, [D, H], "C1x")
            C1h = load_w(c1h, [H, H], "C1h")
            W2x = load_w(w2x, [H, 2 * H], "W2x")
            W2h = load_w(w2h, [H, 2 * H], "W2h")
            C2x = load_w(c2x, [H, H], "C2x")
            C2h = load_w(c2h, [H, H], "C2h")
            GB1 = load_w(gb1n, [128, 2], "GB1", col=True, dty=dt)
            CB1 = load_w(cb1, [H, 1], "CB1", col=True, dty=dt)
            GB2 = load_w(gb2n, [128, 2], "GB2", col=True, dty=dt)
            CB2 = load_w(cb2, [H, 1], "CB2", col=True, dty=dt)
            WQ = load_w(wq, [D, H], "WQ")
            BQ = load_w(bq, [H, 1], "BQ", col=True, dty=dt)
            PRA = load_w(pra, [H, 1], "PRA", col=True, dty=dt)
            PRA1M = load_w(pra1m, [H, 1], "PRA1M", col=True, dty=dt)
            W1APC = load_w(w1apc, [H, 80], "W1APC")
            W1BMC = load_w(w1bmc, [H, 80], "W1BMC")
            W1D = load_w(w1d, [H, 80], "W1D")
            B1 = load_w(b1, [80, 1], "B1", col=True, dty=dt)
            W2A = load_w(w2a, [80, 40], "W2A")
            B2 = load_w(b2, [40, 1], "B2", col=True, dty=dt)
            W3 = load_w(w3, [40, 1], "W3")
            IDN = load_w(ident, [128, 128], "IDN", dty=dt)
            IDNC = load_w(identc, [128, 128], "IDNC")
            NBIG = load_w(negbig, [1, 128], "NBIG")
            ONEC = load_w(onescol, [1, 128], "ONEC")

            QT = wp.tile([D, BL_], dtc, tag="QT", name="QT")
            nc.sync.dma_start(QT[:], qT)
            nbh = (BL_ + 127) // 128
            VAL, NEGM = [], []
            for i in range(nbh):
                p = min(128, BL_ - i * 128)
                v_ = wp.tile([128, T_], dt, tag=f"VAL{i}", name=f"VAL{i}")
                nc.sync.dma_start(v_[0:p, :], validBT[i * 128 : i * 128 + p, :])
                VAL.append(v_)
                n_ = wp.tile([128, T_], dt, tag=f"NEGM{i}", name=f"NEGM{i}")
                nc.sync.dma_start(n_[0:p, :], negmBT[i * 128 : i * 128 + p, :])
                NEGM.append(n_)

            # ---- attention query path (once) ----
            with (
                tc.tile_pool(name="ps_small", bufs=2, space="PSUM") as psp,
                tc.tile_pool(name="setup_tmp", bufs=2) as stp,
            ):
                p_qp = psp.tile([H, BL_], dt, tag="p_qp")
                nc.tensor.matmul(p_qp[:], WQ[:], QT[:], start=True, stop=True)
                qpre = stp.tile([H, BL_], dt, tag="qpre")
                nc.scalar.add(qpre[:], p_qp[:], BQ[:, 0:1])
                r1 = stp.tile([H, BL_], dt, tag="r1")
                nc.scalar.activation(r1[:], qpre[:], RELU, scale=PRA1M[:, 0:1])
                QP = wp.tile([H, BL_], dtc, tag="QP", name="QP")
                nc.vector.scalar_tensor_tensor(
                    QP[:], qpre[:], PRA[:, 0:1], r1[:], op0=MUL, op1=ADDOP
                )
                p_qc = psp.tile([80, BL_], dt, tag="p_qc")
                nc.tensor.matmul(p_qc[:], W1APC[:], QP[:], start=True, stop=True)
                QC = wp.tile([80, BL_], dt, tag="QC", name="QC")
                nc.scalar.add(QC[:], p_qc[:], B1[:, 0:1])

            # =================== GRU recurrences ===================
            def gru_pass(W_x, W_h, C_x, C_h, GBn, CBc, x_of_t, store_rnn1,
                         use_alpha, hout=None):
                with (
                    tc.tile_pool(name="g_x", bufs=4) as xp,
                    tc.tile_pool(name="g_h", bufs=3) as hp,
                    tc.tile_pool(name="g_rv", bufs=2) as rvp,
                    tc.tile_pool(name="g_tmp", bufs=4) as tp,
                    tc.tile_pool(name="g_dead", bufs=3) as dp,
                    tc.tile_pool(name="g_pg", bufs=2, space="PSUM") as pgp,
                    tc.tile_pool(name="g_pc", bufs=1, space="PSUM") as pcp,
                    tc.tile_pool(name="g_pa", bufs=1, space="PSUM") as pap,
                ):
                    NG = 2 if BL_ % 2 == 0 else 1  # pipeline groups (batch halves)
                    GW = BL_ // NG
                    hs_ = []
                    for g in range(NG):
                        h_g = hp.tile([H, GW], dtc, tag=f"h{g}", name=f"h{g}")
                        nc.vector.memset(h_g[:], 0.0)
                        hs_.append(h_g)
                    dead_ch = None
                    alpha_ch = None
                    for t in range(T_):
                        j = t % DCH
                        if j == 0:
                            n_in = min(DCH, T_ - t)
                            dead_ch = dp.tile([1, DCH * BL_], dtc, tag="dead")
                            nc.sync.dma_start(
                                dead_ch[0:1, 0 : n_in * BL_],
                                deadT[t : t + n_in, :].rearrange(
                                    "(c a) b -> c (a b)", c=1),
                            )
                            if use_alpha:
                                alpha_ch = dp.tile([1, DCH * BL_], dtc, tag="alpha")
                                nc.sync.dma_start(
                                    alpha_ch[0:1, 0 : n_in * BL_],
                                    aTd[t : t + n_in, :].rearrange(
                                        "(c a) b -> c (a b)", c=1),
                                )
                        x_t = x_of_t(xp, t)
                        for g in range(NG):
                            lo, hi = g * GW, (g + 1) * GW
                            h = hs_[g]
                            xg = x_t[:, lo:hi]
                            # gates psum [128, 2GW] = [r | v]
                            pg = pgp.tile([128, 2 * GW], dt, tag=f"pg{g}",
                                          name=f"pg{g}")
                            nc.tensor.matmul(pg[:, 0:GW], W_x[:, 0:H], xg,
                                             start=True, stop=False)
                            nc.tensor.matmul(pg[:, 0:GW], W_h[:, 0:H], h[:],
                                             start=False, stop=True)
                            nc.tensor.matmul(pg[:, GW:], W_x[:, H:], xg,
                                             start=True, stop=False)
                            nc.tensor.matmul(pg[:, GW:], W_h[:, H:], h[:],
                                             start=False, stop=False)
                            nc.tensor.matmul(
                                pg[:, GW:], NBIG[:],
                                dead_ch[0:1, j * BL_ + lo : j * BL_ + hi],
                                start=False, stop=True)
                            rv = rvp.tile([128, 2 * GW], dtc, tag=f"rv{g}",
                                          name=f"rv{g}")
                            nc.scalar.activation(rv[:, 0:GW], pg[:, 0:GW], SIG,
                                                 bias=GBn[:, 0:1])
                            nc.scalar.activation(rv[:, GW:], pg[:, GW:], SIG,
                                                 bias=GBn[:, 1:2])
                            rh = tp.tile([H, GW], dtc, tag=f"rh{g}", name=f"rh{g}")
                            nc.vector.tensor_mul(rh[:], rv[:, 0:GW], h[:])
                            pc = pcp.tile([H, GW], dt, tag=f"pc{g}", name=f"pc{g}")
                            nc.tensor.matmul(pc[:], C_x[:], xg,
                                             start=True, stop=False)
                            nc.tensor.matmul(pc[:], C_h[:], rh[:],
                                             start=False, stop=True)
                            c = tp.tile([H, GW], dtc, tag=f"c{g}", name=f"c{g}")
                            nc.scalar.activation(c[:], pc[:], TANH, bias=CBc[:, 0:1])
                            if use_alpha:
                                # pa = bcast of abar=(1-alpha_t); u1 = (v-1)*abar
                                pa = pap.tile([128, GW], dt, tag=f"pa{g}",
                                              name=f"pa{g}")
                                nc.tensor.matmul(
                                    pa[:], ONEC[:],
                                    alpha_ch[0:1, j * BL_ + lo : j * BL_ + hi],
                                    start=True, stop=True)
                                u1 = tp.tile([H, GW], dtc, tag=f"u1{g}",
                                             name=f"u1{g}")
                                nc.vector.scalar_tensor_tensor(
                                    u1[:], rv[:, GW:], 1.0, pa[:],
                                    op0=SUB, op1=MUL)
                                p_ = tp.tile([H, GW], dtc, tag=f"p_{g}",
                                             name=f"p_{g}")
                                nc.gpsimd.tensor_mul(p_[:], u1[:], h[:])
                                vc = tp.tile([H, GW], dtc, tag=f"vc{g}",
                                             name=f"vc{g}")
                                nc.vector.scalar_tensor_tensor(
                                    vc[:], u1[:], 1.0, c[:], op0=ADDOP, op1=MUL)
                            else:
                                # p_ = (v-1)*h ; vc = v*c ; h' = vc - p_
                                p_ = tp.tile([H, GW], dtc, tag=f"p_{g}",
                                             name=f"p_{g}")
                                nc.gpsimd.scalar_tensor_tensor(
                                    p_[:], rv[:, GW:], 1.0, h[:],
                                    op0=SUB, op1=MUL)
                                vc = tp.tile([H, GW], dtc, tag=f"vc{g}",
                                             name=f"vc{g}")
                                nc.vector.tensor_mul(vc[:], rv[:, GW:], c[:])
                            h2 = hp.tile([H, GW], dtc, tag=f"h{g}", name=f"h{g}")
                            nc.vector.tensor_sub(h2[:], vc[:], p_[:])
                            if store_rnn1:
                                nc.sync.dma_start(rnn1[t][:, lo:hi], h2[:])
                            hs_[g] = h2
                    if hout is not None:
                        for g in range(NG):
                            nc.vector.tensor_copy(
                                hout[:, g * GW : (g + 1) * GW], hs_[g][:])

            XB = 4
            xT_p = xT.rearrange("t p b -> p t b")
            xcache = {}

            def x_from_xT(xp, t):
                t0 = (t // XB) * XB
                if t0 not in xcache:
                    nb = min(XB, T_ - t0)
                    xc = xp.tile([D, XB * BL_], dtc, tag="x", name=f"x{t0}")
                    nc.sync.dma_start(
                        xc[:].rearrange("p (a b) -> p a b", a=XB)[:, 0:nb, :],
                        xT_p[:, t0 : t0 + nb, :])
                    xcache.clear()
                    xcache[t0] = xc
                return xcache[t0][:, (t - t0) * BL_ : (t - t0 + 1) * BL_]

            hfin = wp.tile([H, BL_], dtc, tag="hfin", name="hfin")
            gru_pass(W1x, W1h, C1x, C1h, GB1, CB1, x_from_xT, True, False)

            # =================== attention scores ===================
            rnn1_p = rnn1.rearrange("t p b -> p t b")
            TCH = max(1, 512 // BL_)
            n_tch = (T_ + TCH - 1) // TCH
            with (
                tc.tile_pool(name="a_sc", bufs=1, space="PSUM") as scp,
                tc.tile_pool(name="a_in", bufs=4) as ainp,
                tc.tile_pool(name="a_tmp", bufs=4) as atp,
            ):
                p_sc = [
                    scp.tile([128, T_], dt, tag=f"p_sc{i}", name=f"p_sc{i}")
                    for i in range(nbh)
                ]
                app_cm = tc.tile_pool(name="a_ps", bufs=2, space="PSUM")
                app = app_cm.__enter__()
                for ci in range(n_tch):
                    t0 = ci * TCH
                    nt = min(TCH, T_ - t0)
                    w_ = nt * BL_
                    rc = ainp.tile([H, TCH * BL_], dtc, tag="rc")
                    rc3 = rc[:].rearrange("p (a b) -> p a b", a=TCH)[:, 0:nt, :]
                    nc.sync.dma_start(rc3, rnn1_p[:, t0 : t0 + nt, :])
                    qp_bc = (
                        QP[:].rearrange("p (a b) -> p a b", a=1)
                        .broadcast_to([H, nt, BL_])
                    )
                    prod = atp.tile([H, TCH * BL_], dtc, tag="prod")
                    prod3 = prod[:].rearrange("p (a b) -> p a b", a=TCH)[:, 0:nt, :]
                    nc.vector.tensor_mul(prod3, rc3, qp_bc)
                    p_s1 = app.tile([80, TCH * BL_], dt, tag="p_s1")
                    nc.tensor.matmul(p_s1[:, 0:w_], W1BMC[:], rc[:, 0:w_],
                                     start=True, stop=False)
                    nc.tensor.matmul(p_s1[:, 0:w_], W1D[:], prod[:, 0:w_],
                                     start=False, stop=True)
                    qc_bc = (
                        QC[:].rearrange("p (a b) -> p a b", a=1)
                        .broadcast_to([80, nt, BL_])
                    )
                    a1 = atp.tile([80, TCH * BL_], dt, tag="a1")
                    nc.vector.tensor_add(
                        a1[:].rearrange("p (a b) -> p a b", a=TCH)[:, 0:nt, :],
                        p_s1[:].rearrange("p (a b) -> p a b", a=TCH)[:, 0:nt, :],
                        qc_bc,
                    )
                    a1s = atp.tile([80, TCH * BL_], dtc, tag="a1s")
                    nc.scalar.activation(a1s[:, 0:w_], a1[:, 0:w_], SIG)
                    p_s2 = app.tile([40, TCH * BL_], dt, tag="p_s2")
                    nc.tensor.matmul(p_s2[:, 0:w_], W2A[:], a1s[:, 0:w_],
                                     start=True, stop=True)
                    a2 = atp.tile([40, TCH * BL_], dtc, tag="a2")
                    nc.scalar.activation(a2[:, 0:w_], p_s2[:, 0:w_], SIG,
                                         bias=B2[:, 0:1])
                    for k in range(nt):
                        t = t0 + k
                        for i in range(nbh):
                            p = min(128, BL_ - i * 128)
                            nc.tensor.matmul(
                                p_sc[i][0:p, t : t + 1],
                                a2[:, k * BL_ + i * 128 : k * BL_ + i * 128 + p],
                                W3[:], start=True, stop=True)
                app_cm.__exit__(None, None, None)
                # masked softmax; store abar = (1 - alpha) transposed to aTd
                with (
                    tc.tile_pool(name="a_sm", bufs=1) as smp,
                    tc.tile_pool(name="a_tr", bufs=2, space="PSUM") as trp,
                ):
                    for i in range(nbh):
                        p = min(128, BL_ - i * 128)
                        sm = smp.tile([128, T_], dt, tag=f"sm{i}", name=f"sm{i}")
                        nc.vector.scalar_tensor_tensor(
                            sm[0:p, :], p_sc[i][0:p, :], 1.0, VAL[i][0:p, :],
                            op0=MUL, op1=MUL)
                        msk = smp.tile([128, T_], dt, tag=f"msk{i}", name=f"msk{i}")
                        nc.vector.tensor_add(msk[0:p, :], sm[0:p, :], NEGM[i][0:p, :])
                        nmx = smp.tile([128, 1], dt, tag=f"nmx{i}", name=f"nmx{i}")
                        nc.vector.tensor_reduce(
                            nmx[0:p, :], msk[0:p, :], axis=AX, op=MAXOP, negate=True)
                        ex = smp.tile([128, T_], dt, tag=f"ex{i}", name=f"ex{i}")
                        nc.scalar.activation(ex[0:p, :], msk[0:p, :], EXP,
                                             bias=nmx[0:p, 0:1])
                        sume = smp.tile([128, 1], dt, tag=f"sume{i}", name=f"sume{i}")
                        nc.vector.tensor_reduce(
                            sume[0:p, :], ex[0:p, :], axis=AX, op=ADDOP)
                        rec = smp.tile([128, 1], dt, tag=f"rec{i}", name=f"rec{i}")
                        nc.vector.reciprocal(rec[0:p, :], sume[0:p, :])
                        alp = smp.tile([128, T_], dt, tag=f"alp{i}", name=f"alp{i}")
                        nc.vector.tensor_scalar_mul(alp[0:p, :], ex[0:p, :],
                                                    rec[0:p, 0:1])
                        for c0 in range(0, T_, 128):
                            w2_ = min(128, T_ - c0)
                            pt = trp.tile([128, 128], dt, tag="p_tr")
                            nc.tensor.transpose(
                                pt[0:w2_, 0:p], alp[0:p, c0 : c0 + w2_],
                                IDN[0:p, 0:p])
                            st = smp.tile([128, 128], dtc, tag="st", name="st")
                            # abar = 1 - alpha, cast to compute dtype
                            nc.scalar.activation(
                                st[0:w2_, 0:p], pt[0:w2_, 0:p], COPYF,
                                bias=1.0, scale=-1.0)
                            nc.sync.dma_start(
                                aTd[c0 : c0 + w2_, i * 128 : i * 128 + p],
                                st[0:w2_, 0:p])

            # =================== AUGRU ===================
            rcache = {}

            def x_from_rnn1(xp, t):
                t0 = (t // XB) * XB
                if t0 not in rcache:
                    nb = min(XB, T_ - t0)
                    xc = xp.tile([H, XB * BL_], dtc, tag="x", name=f"xr{t0}")
                    nc.sync.dma_start(
                        xc[:].rearrange("p (a b) -> p a b", a=XB)[:, 0:nb, :],
                        rnn1_p[:, t0 : t0 + nb, :])
                    rcache.clear()
                    rcache[t0] = xc
                return rcache[t0][:, (t - t0) * BL_ : (t - t0 + 1) * BL_]

            gru_pass(W2x, W2h, C2x, C2h, GB2, CB2, x_from_rnn1, False, True,
                     hout=hfin)

            # =================== output assembly ===================
            with (
                tc.tile_pool(name="o_t", bufs=2) as otp,
                tc.tile_pool(name="o_p", bufs=2, space="PSUM") as opp,
            ):
                for i in range(nbh if parts == "all" else 0):
                    p = min(128, BL_ - i * 128)
                    qs = otp.tile([128, D], dt, tag="qs")
                    nc.sync.dma_start(qs[0:p, :], qN[i * 128 : i * 128 + p, :])
                    hs = otp.tile([128, D], dt, tag="hs")
                    nc.sync.dma_start(hs[0:p, :], hsum[i * 128 : i * 128 + p, :])
                    pr = otp.tile([128, D], dt, tag="pr")
                    nc.vector.tensor_mul(pr[0:p, :], qs[0:p, :], hs[0:p, :])
                    nc.sync.dma_start(out[i * 128 : i * 128 + p, 0:D], qs[0:p, :])
                    nc.sync.dma_start(out[i * 128 : i * 128 + p, D : 2 * D], hs[0:p, :])
                    nc.sync.dma_start(out[i * 128 : i * 128 + p, 2 * D : 3 * D],
                                      pr[0:p, :])
                    ptr = opp.tile([128, 128], dtc, tag="ptr")
                    nc.tensor.transpose(ptr[0:p, :], hfin[:, i * 128 : i * 128 + p],
                                        IDNC[:])
                    ht = otp.tile([128, H], dt, tag="ht")
                    nc.scalar.copy(ht[0:p, :], ptr[0:p, :])
                    nc.sync.dma_start(out[i * 128 : i * 128 + p, 3 * D :], ht[0:p, :])

    nc.compile()
    return nc


def host_prep(item_eb, item_his_eb, item_his_eb_sum, mask,
              gk1, gb1, ck1, cb1,
              wq, bq, prelu_alpha, w1, b1, w2, b2, w3, b3,
              gk2, gb2, ck2, cb2, T_=T, BL_=BL, ncores=NCORES, dtc_name="bf16"):
    f = np.float32
    fc = ml_dtypes.bfloat16 if dtc_name == "bf16" else np.float32

    w1x = np.ascontiguousarray(gk1[:D]).astype(f)
    w1h = np.ascontiguousarray(gk1[D:]).astype(f)
    w1x[:, H:] = -w1x[:, H:]
    w1h[:, H:] = -w1h[:, H:]
    gb1n = np.asarray(gb1, f).copy()
    gb1n[H:] = -gb1n[H:]
    w2x_ = np.ascontiguousarray(gk2[:H]).astype(f)
    w2h_ = np.ascontiguousarray(gk2[H:]).astype(f)
    w2x_[:, H:] = -w2x_[:, H:]
    w2h_[:, H:] = -w2h_[:, H:]
    gb2n = np.asarray(gb2, f).copy()
    gb2n[H:] = -gb2n[H:]
    shared = dict(
        w1x=w1x.astype(fc), w1h=w1h.astype(fc),
        c1x=np.ascontiguousarray(ck1[:D]).astype(fc),
        c1h=np.ascontiguousarray(ck1[D:]).astype(fc),
        gb1n=gb1n, cb1=np.asarray(cb1, f),
        w2x=w2x_.astype(fc), w2h=w2h_.astype(fc),
        c2x=np.ascontiguousarray(ck2[:H]).astype(fc),
        c2h=np.ascontiguousarray(ck2[H:]).astype(fc),
        gb2n=gb2n, cb2=np.asarray(cb2, f),
        wq=np.asarray(wq).astype(fc), bq=np.asarray(bq, f),
        pra=np.asarray(prelu_alpha, f),
        pra1m=(1.0 - np.asarray(prelu_alpha, f)),
        w1apc=np.ascontiguousarray(w1[:H] + w1[2 * H : 3 * H]).astype(fc),
        w1bmc=np.ascontiguousarray(w1[H : 2 * H] - w1[2 * H : 3 * H]).astype(fc),
        w1d=np.ascontiguousarray(w1[3 * H :]).astype(fc),
        b1=np.asarray(b1, f), w2a=np.asarray(w2).astype(fc),
        b2=np.asarray(b2, f), w3=np.asarray(w3).astype(fc),
        ident=np.eye(128, dtype=f), identc=np.eye(128).astype(fc),
        negbig=np.full((1, 128), -BIG).astype(fc),
        onescol=np.ones((1, 128)).astype(fc),
    )
    in_maps = []
    for c in range(ncores):
        sl = slice(c * BL_, (c + 1) * BL_)
        m = np.asarray(mask[sl])
        has0 = (m == 0).any(axis=1)
        ln = np.where(has0, np.argmax(m == 0, axis=1), T_).astype(np.int64)
        tt = np.arange(T_)
        valid = tt[None, :] < ln[:, None]
        im = dict(shared)
        im["xT"] = np.ascontiguousarray(
            np.asarray(item_his_eb[sl]).transpose(1, 2, 0)).astype(fc)
        im["qT"] = np.ascontiguousarray(np.asarray(item_eb[sl, 0]).T).astype(fc)
        im["qN"] = np.ascontiguousarray(np.asarray(item_eb[sl, 0])).astype(f)
        im["hsum"] = np.ascontiguousarray(np.asarray(item_his_eb_sum[sl])).astype(f)
        im["validBT"] = valid.astype(f)
        im["negmBT"] = np.where(valid, 0.0, NEG).astype(f)
        im["deadT"] = np.ascontiguousarray((~valid).T).astype(fc)
        in_maps.append(im)
    return in_maps


_prog_cache = {}


def kernel(**inputs):
    key = "full"
    if key not in _prog_cache:
        _prog_cache[key] = build_program()
    nc = _prog_cache[key]
    in_maps = host_prep(**inputs)
    res = run_bass_kernel_spmd(nc, in_maps, list(range(NCORES)))
    return np.concatenate([res.results[c]["out"] for c in range(NCORES)], axis=0)
